# revision 13
# baseline (speedup 1.0000x reference)
"""Clustered-attention Trainium2 kernel (Bass/Tile), 8-core SPMD.

Problem (per batch b, variable k, with L=512, V=32, D=64, C=8 clusters):
    S   = sum_v key[b,:,v,:]                  # (L, D) shared key-sum
    sc  = query[b,:,k,:] @ S.T / sqrt(D)      # (L, L)
    sc  = where(label[i]==label[j], sc, -inf)
    out = softmax(sc, -1) @ value[b,:,k,:]

Layout strategy:
  - Host FFD-packs the 8 clusters of each batch into NB=5 bins of <=128
    rows (zero-padded).  Every cluster lies entirely inside one bin, so
    the score matrix is block-diagonal over bins: only NB 128x128
    diagonal blocks are computed per head (vs 10 chunk-pairs in a
    sorted-chunk scheme).
  - The cluster mask folds into the matmul: 8 one-hot label rows scaled
    8*B (B=96) on the lhsT side, 1.0 on the rhs side; exp(z/8 - B) is
    exp(q.s/8) for same-cluster pairs and ~e^-96+25 otherwise.
  - Work unit = (bin, 8-head group).  4 groups x NB bins = 20 units per
    batch, split 10/10 between the two cores of the batch so each core
    touches only ceil(NB/2)+... = 3 bins -> loads 3/5 of the key.
  - Per unit: 2 score matmuls [72,4,128] -> psum [128,8,128], ONE exp
    over [128,1024] (head-batched to amortize the ~150ns activation
    overhead), 8 A@V matmuls with a ones-column appended to V so the
    softmax denominator lands in psum col 64.  Normalization happens on
    the host (num/den), outputs ship as bf16.
  - keysum via fp16 tree adds on DVE per bin-slot; S^T per slot via PE
    transpose.  Fat 128-partition DMAs (one per tensor/slot).

Host work is only permute/pad/cast/divide (the final num/den divide).
"""

import numpy as np

import concourse.bass as bass
import concourse.tile as tile
from concourse import mybir
from concourse.masks import make_identity
from concourse.tile import TileContext, ScopedClock

B, L, V, D = 4, 512, 32, 64
NC = 8  # cores
NCLUST = 8
BIAS = 96.0
F32 = mybir.dt.float32
F16 = mybir.dt.float16
BF16 = mybir.dt.bfloat16

PROFILE = False
LAST_RESULT = None

_PATCHED = False


def _patch_tile_drain():
    """Walrus on this image rejects multiple sync-waits on one instruction
    ("Too many sync wait commands"). Legalize by splitting surplus waits
    onto NoOp instructions inserted just before, on the same engine."""
    global _PATCHED
    if _PATCHED:
        return
    _PATCHED = True

    _orig_add = TileContext._add_instruction

    def _add_instruction(self, inst):
        si = getattr(inst, "sync_info", None)
        if (
            si is not None
            and si.on_wait
            and len(si.on_wait) > 1
            and inst.engine != mybir.EngineType.Unassigned
        ):
            waits = list(si.on_wait)
            for w in waits[:-1]:
                nop = mybir.InstNoOp(name=self.nc.get_next_instruction_name())
                nop.engine = inst.engine
                nop.sync_info = mybir.SyncInfo(on_wait=[w], on_update=[])
                _orig_add(self, nop)
            inst.sync_info = mybir.SyncInfo(
                on_wait=[waits[-1]], on_update=list(si.on_update or [])
            )
        _orig_add(self, inst)

    TileContext._add_instruction = _add_instruction

    def _drain_and_barrier(self, tick_clock, wait_clock):
        nc = self.nc
        drain_inst = nc.sync.drain()
        wait_clock.add_sem_waits(
            drain_inst.ins, ScopedClock({None: tick_clock.global_clock})
        )
        si = drain_inst.ins.sync_info
        if si is not None and si.on_wait and len(si.on_wait) > 1:
            waits = list(si.on_wait)
            drain_inst.ins.sync_info = mybir.SyncInfo(
                on_wait=waits[:1], on_update=list(si.on_update or [])
            )
            for i in range(1, len(waits)):
                nop = nc.sync.nop(nofuse=True, hint=f"drain_split_{i}")
                nop.ins.sync_info = mybir.SyncInfo(on_wait=[waits[i]], on_update=[])
        nc.all_engine_barrier()
        assert self.sems is not None
        popped = nc._tile_sem_poison_stack.pop()
        assert popped is self._sem_poison
        nc.clear_and_free_semaphores(list(self.sems.allocated().values()))
        nc.all_engine_barrier()
    TileContext._drain_and_barrier = _drain_and_barrier


def _build_nc(nslot, nu, slotmap):
    """nslot: bin-slots this core holds; nu: units (slot, 8-head group);
    slotmap[u] -> slot index. Uniform across cores (data differs)."""
    nc = bass.Bass("TRN2", target_bir_lowering=False, debug=False)

    kin = nc.dram_tensor("kin", [128, nslot * V * D], F16, kind="ExternalInput").ap()
    qin = nc.dram_tensor("qin", [D, nu * 8 * 128], F16, kind="ExternalInput").ap()
    vin = nc.dram_tensor("vin", [128, nu * 8 * 66], BF16, kind="ExternalInput").ap()
    lab = nc.dram_tensor("lab", [1, nslot * 128], F32, kind="ExternalInput").ap()
    iota8 = nc.dram_tensor("iota8", [8, 1], F32, kind="ExternalInput").ap()
    o_out = nc.dram_tensor("o", [128, nu * 8 * 65], BF16, kind="ExternalOutput").ap()

    with TileContext(nc) as tc:
        with (
            tc.tile_pool(name="singles", bufs=1) as singles,
            tc.tile_pool(name="redpool", bufs=2) as redpool,
            tc.tile_pool(name="epool", bufs=3) as epool,
            tc.tile_pool(name="ocpool", bufs=3) as ocpool,
            tc.tile_pool(name="ps_score", bufs=2, space="PSUM") as ps_score,
            tc.tile_pool(name="ps_u", bufs=4, space="PSUM") as ps_u,
        ):
            # ---- constants (no DMA deps) ----
            identity = singles.tile([128, 128], F16)
            make_identity(nc, identity)
            negb = singles.tile([128, 1], F32)
            nc.vector.memset(negb, -BIAS)
            dummy = singles.tile([128, 1], F32)
            nc.scalar.activation(dummy, negb, mybir.ActivationFunctionType.Exp)
            junk = singles.tile([128, 512], F16)
            nc.vector.memset(junk, 1.0)

            # ---- bulk DMAs: few fat 128-partition transfers.
            # lab goes first (tiny; gates the one-hot chain), then key
            # slots chained (slot s+1 starts after slot s lands) so the
            # first tree starts ASAP; q/v pieces trail the key slot they
            # pair with in consumption order. ----
            lab_sb = singles.tile([8, nslot * 128], F32)
            lab_bcast = bass.AP(tensor=lab.tensor, offset=lab.offset,
                                ap=[[0, 8]] + list(lab.ap[1:]))
            nc.sync.dma_start(out=lab_sb, in_=lab_bcast)
            iota_sb = singles.tile([8, 1], F32)
            nc.gpsimd.iota(iota_sb, [[0, 1]], channel_multiplier=1,
                           allow_small_or_imprecise_dtypes=True)

            kcs = []
            kdmas = []
            for s in range(nslot):
                kc = singles.tile([128, V * D], F16, tag=f"kc{s}", name=f"kc{s}")
                d = nc.sync.dma_start(out=kc, in_=kin[:, s * V * D:(s + 1) * V * D])
                if kdmas:
                    tile.add_dep_helper(d.ins, kdmas[-1].ins,
                                        reason="key slot chain")
                kcs.append(kc)
                kdmas.append(d)

            qtile = singles.tile([D + 8, nu * 8 * 128], F16)
            vtile = singles.tile([128, nu * 8 * 66], BF16)
            # q/v pieces paced with the key slots: units 0-1 after slot 0,
            # units 2-5 after slot 1, units 6+ after slot 2.
            qsplit = [(0, 2, 0), (2, 6, 1), (6, nu, min(2, nslot - 1))]
            for lo, hi, ks in qsplit:
                qd = nc.gpsimd.dma_start(
                    out=qtile[0:D, lo * 8 * 128:hi * 8 * 128],
                    in_=qin[:, lo * 8 * 128:hi * 8 * 128])
                tile.add_dep_helper(qd.ins, kdmas[ks].ins,
                                    reason=f"q{lo}-{hi} after ks{ks}")
                vd = nc.gpsimd.dma_start(
                    out=vtile[:, lo * 8 * 66:hi * 8 * 66],
                    in_=vin[:, lo * 8 * 66:hi * 8 * 66])
                tile.add_dep_helper(vd.ins, kdmas[ks].ins,
                                    reason=f"v{lo}-{hi} after ks{ks}")

            # ---- one-hot labels ----
            onehot = singles.tile([8, nslot * 128], F32)
            nc.vector.tensor_scalar(onehot, lab_sb, iota_sb, None,
                                    op0=mybir.AluOpType.is_equal)
            oh16 = singles.tile([8, nslot * 128], F16)
            nc.vector.tensor_copy(oh16, onehot)

            # ---- stb tiles: [S^T ; 8B*onehot] per slot ----
            stbs = []
            for s in range(nslot):
                stb = singles.tile([D + 8, 128], F16, tag=f"stb{s}",
                                   name=f"stb{s}")
                nc.vector.tensor_scalar_mul(
                    stb[D:D + 8, :], onehot[:, s * 128:(s + 1) * 128],
                    8.0 * BIAS)
                stbs.append(stb)

            # ---- PE warmup ----
            for w in range(4):
                wps = ps_score.tile([128, 512], F32, tag="ps", name=f"warm{w}")
                nc.tensor.matmul(wps, lhsT=identity, rhs=junk,
                                 start=True, stop=True)

            # ---- keysum per slot: fp16 tree -> S_s [128, D]; PE
            # transpose -> stb rows 0:D ----
            def _make_stb(s):
                cur = kcs[s]
                width = V * D
                while width > D:
                    width //= 2
                    nxt = redpool.tile([128, width], F16, tag=f"red{width}",
                                       name=f"red{s}_{width}")
                    nc.vector.tensor_tensor(
                        out=nxt, in0=cur[:, 0:width],
                        in1=cur[:, width:2 * width],
                        op=mybir.AluOpType.add)
                    cur = nxt
                st_ps = ps_score.tile([64, 128], F16, tag="ps",
                                      name=f"st{s}")
                nc.tensor.transpose(st_ps, cur, identity)
                nc.vector.tensor_copy(stbs[s][0:D, :], st_ps)

            # slot-0 tree FIRST in the DVE queue (it gates the first
            # scores); the one-hot broadcast below is off the critical
            # path and sits behind it.
            _make_stb(0)
            made = 1

            # q one-hot rows: broadcast each slot's one-hot over that
            # slot's units and their 8 heads via SBUF->SBUF DMA on the
            # Vector ring (fabric path; keeps DVE itself free).
            u = 0
            while u < nu:
                s = slotmap[u]
                span = 1
                while u + span < nu and slotmap[u + span] == s:
                    span += 1
                dst = qtile[D:D + 8, u * 8 * 128:(u + span) * 8 * 128]
                dst3 = dst.rearrange("p (x i) -> p x i", x=span * 8, i=128)
                src = oh16[:, s * 128:(s + 1) * 128]
                src_b = bass.AP(tensor=src.tensor, offset=src.offset,
                                ap=[list(src.ap[0]), [0, span * 8],
                                    list(src.ap[1])])
                nc.scalar.dma_start(out=dst3, in_=src_b)
                u += span

            # ---- unit pipeline ----
            qv = qtile.rearrange("p (u h i) -> p u h i", u=nu, h=8, i=128)
            vv = vtile.rearrange("p (u h c) -> p u h c", u=nu, h=8, c=66)
            ov = o_out.rearrange("p (u h c) -> p u h c", u=nu, h=8, c=65)

            def _ensure_stb(s):
                nonlocal made
                while made <= s:
                    _make_stb(made)
                    made += 1

            def _scores(u):
                s = slotmap[u]
                _ensure_stb(s)
                ps = ps_score.tile([128, 8, 128], F32, tag="ps",
                                   name=f"ps{u}")
                nc.tensor.matmul(ps[:, 0:4, :], lhsT=stbs[s],
                                 rhs=qv[:, u, 0:4, :], start=True, stop=True)
                nc.tensor.matmul(ps[:, 4:8, :], lhsT=stbs[s],
                                 rhs=qv[:, u, 4:8, :], start=True, stop=True)
                e_t = epool.tile([128, 8, 128], BF16, tag="et", name=f"et{u}")
                nc.scalar.activation(
                    e_t.rearrange("p a b -> p (a b)"),
                    ps.rearrange("p a b -> p (a b)"),
                    mybir.ActivationFunctionType.Exp,
                    bias=negb, scale=1.0 / 8.0)
                return e_t

            def _tail(u, e_t):
                psus = []
                for half in range(2):
                    psu = ps_u.tile([128, 4, 65], F32, tag="psu",
                                    name=f"psu{u}_{half}")
                    for hh in range(4):
                        h = half * 4 + hh
                        nc.tensor.matmul(
                            psu[:, hh, :], lhsT=e_t[:, h, :],
                            rhs=vv[:, u, h, 0:65], start=True, stop=True)
                    psus.append(psu)
                oc = ocpool.tile([128, 8, 65], BF16, tag="oc", name=f"oc{u}")
                for half in range(2):
                    nc.vector.tensor_copy(oc[:, half * 4:half * 4 + 4, :],
                                          psus[half])
                nc.scalar.dma_start(out=ov[:, u], in_=oc)

            prev = None
            for u in range(nu):
                e_t = _scores(u)
                if prev is not None:
                    _tail(*prev)
                prev = (u, e_t)
            _tail(*prev)
    return nc


_NC_CACHE = {}


def _get_nc(nslot, nu, slotmap):
    key = (nslot, nu, tuple(slotmap))
    if key not in _NC_CACHE:
        _patch_tile_drain()
        _NC_CACHE[key] = _build_nc(nslot, nu, slotmap)
    return _NC_CACHE[key]


def _pack_bins(sizes):
    """FFD pack cluster sizes into bins of <=128. Returns list of bins,
    each a list of cluster ids."""
    order = np.argsort(sizes)[::-1]
    bins, fill = [], []
    for c in order:
        placed = False
        for i in range(len(bins)):
            if fill[i] + sizes[c] <= 128:
                bins[i].append(int(c))
                fill[i] += sizes[c]
                placed = True
                break
        if not placed:
            bins.append([int(c)])
            fill.append(int(sizes[c]))
    return bins


def kernel(query, key, value, label_arr):
    """Full inputs (B,L,V,D)/(B,L) -> full output (B,L,V,D)."""
    global LAST_RESULT
    import ml_dtypes
    from concourse.bass_utils import run_bass_kernel_spmd

    query = np.asarray(query, dtype=np.float32)
    key = np.asarray(key, dtype=np.float32)
    value = np.asarray(value, dtype=np.float32)
    labels = np.asarray(label_arr)
    iota = np.arange(8, dtype=np.float32).reshape(8, 1)

    # per-batch bin packing (identical structure needed across batches)
    batch_bins = []
    for b in range(B):
        sizes = np.bincount(labels[b], minlength=NCLUST)
        assert sizes.max() <= 128, "cluster larger than 128 rows unsupported"
        batch_bins.append(_pack_bins(sizes))
    nb = max(len(bb) for bb in batch_bins)
    # units: (bin, head-group) with 4 groups of 8 heads; split 2*nb/2*nb
    # between the two cores of a batch with a uniform slot pattern.
    units_all = [(bi, g) for bi in range(nb) for g in range(4)]
    half0, half1 = units_all[:2 * nb], units_all[2 * nb:]
    if nb % 2:
        # reorder half1: full bins first, shared bin's tail groups last,
        # so both halves follow the same slot pattern (e.g. 4,4,2).
        shared = [un for un in half1 if un[0] == nb // 2]
        full = [un for un in half1 if un[0] != nb // 2]
        half1 = full + shared
    nu = 2 * nb
    # slots = unique bins in core order; slotmap uniform across halves
    def mkslots(units):
        slots, smap = [], []
        for bi, _g in units:
            if bi not in slots:
                slots.append(bi)
            smap.append(slots.index(bi))
        return slots, smap
    slots0, smap0 = mkslots(half0)
    slots1, smap1 = mkslots(half1)
    assert smap0 == smap1, (smap0, smap1)
    nslot = len(slots0)

    in_maps = []
    core_meta = []
    for c in range(NC):
        b = c // 2
        bins = batch_bins[b]
        # rows of each bin (original indices), padded to 128
        order = np.argsort(labels[b], kind="stable")
        labs_sorted = labels[b][order]
        cluster_rows = {cl: order[labs_sorted == cl] for cl in range(NCLUST)}
        binrows, binvalid = [], []
        for bb in bins:
            rows = np.concatenate([cluster_rows[cl] for cl in bb])
            w = len(rows)
            rp = np.zeros(128, dtype=np.int64)
            rp[:w] = rows
            binrows.append(rp)
            binvalid.append(w)
        while len(binrows) < nb:  # batches that packed into fewer bins
            binrows.append(np.zeros(128, dtype=np.int64))
            binvalid.append(0)

        units = half0 if c % 2 == 0 else half1
        slots = slots0 if c % 2 == 0 else slots1

        karr = np.zeros((128, nslot, V, D), dtype=np.float16)
        labarr = np.full((1, nslot * 128), -1.0, dtype=np.float32)
        for si, bi in enumerate(slots):
            w = binvalid[bi]
            if w:
                karr[:w, si] = key[b][binrows[bi][:w]]
                labarr[0, si * 128:si * 128 + w] = labels[b][binrows[bi][:w]]

        qarr = np.zeros((D, nu, 8, 128), dtype=np.float16)
        varr = np.zeros((128, nu, 8, 66), dtype=ml_dtypes.bfloat16)
        for u, (bi, g) in enumerate(units):
            w = binvalid[bi]
            if not w:
                continue
            rows = binrows[bi][:w]
            # (w, 8, D) -> (D, 8, w)
            qarr[:, u, :, :w] = query[b][rows, 8 * g:8 * g + 8, :].transpose(
                2, 1, 0)
            varr[:w, u, :, 0:D] = value[b][rows, 8 * g:8 * g + 8, :]
            varr[:w, u, :, D] = 1.0
        in_maps.append({
            "kin": karr.reshape(128, nslot * V * D),
            "qin": qarr.reshape(D, nu * 8 * 128),
            "vin": varr.reshape(128, nu * 8 * 66),
            "lab": labarr,
            "iota8": iota,
        })
        core_meta.append((b, units, binrows, binvalid))

    nc = _get_nc(nslot, nu, smap0)
    kwargs = {}
    if PROFILE:
        kwargs["trace"] = True
    res = run_bass_kernel_spmd(nc, in_maps, list(range(NC)), **kwargs)
    LAST_RESULT = res

    out = np.empty((B, L, V, D), dtype=np.float32)
    for c in range(NC):
        b, units, binrows, binvalid = core_meta[c]
        o = res.results[c]["o"].reshape(128, nu, 8, 65).astype(np.float32)
        for u, (bi, g) in enumerate(units):
            w = binvalid[bi]
            if not w:
                continue
            rows = binrows[bi][:w]
            num = o[:w, u, :, 0:D]
            den = o[:w, u, :, D:D + 1]
            out[b][rows, 8 * g:8 * g + 8, :] = num / den
    return out


# revision 15
# speedup vs baseline: 1.1054x; 1.1054x over previous
"""Clustered-attention Trainium2 kernel (Bass/Tile), 8-core SPMD.

Problem (per batch b, variable k, with L=512, V=32, D=64, C=8 clusters):
    S   = sum_v key[b,:,v,:]                  # (L, D) shared key-sum
    sc  = query[b,:,k,:] @ S.T / sqrt(D)      # (L, L)
    sc  = where(label[i]==label[j], sc, -inf)
    out = softmax(sc, -1) @ value[b,:,k,:]

Layout strategy:
  - Host FFD-packs the 8 clusters of each batch into NB=5 bins of <=128
    rows (zero-padded).  Every cluster lies entirely inside one bin, so
    the score matrix is block-diagonal over bins: only NB 128x128
    diagonal blocks are computed per head (vs 10 chunk-pairs in a
    sorted-chunk scheme).
  - The cluster mask folds into the matmul: 8 one-hot label rows scaled
    8*B (B=96) on the lhsT side, 1.0 on the rhs side; exp(z/8 - B) is
    exp(q.s/8) for same-cluster pairs and ~e^-96+25 otherwise.
  - Work unit = (bin, 8-head group).  4 groups x NB bins = 20 units per
    batch, split 10/10 between the two cores of the batch so each core
    touches only ceil(NB/2)+... = 3 bins -> loads 3/5 of the key.
  - Per unit: 2 score matmuls [72,4,128] -> psum [128,8,128], ONE exp
    over [128,1024] (head-batched to amortize the ~150ns activation
    overhead), 8 A@V matmuls with a ones-column appended to V so the
    softmax denominator lands in psum col 64.  Normalization happens on
    the host (num/den), outputs ship as bf16.
  - keysum via fp16 tree adds on DVE per bin-slot; S^T per slot via PE
    transpose.  Fat 128-partition DMAs (one per tensor/slot).

Host work is only permute/pad/cast/divide (the final num/den divide).
"""

import numpy as np

import concourse.bass as bass
import concourse.tile as tile
from concourse import mybir
from concourse.masks import make_identity
from concourse.tile import TileContext, ScopedClock

B, L, V, D = 4, 512, 32, 64
NC = 8  # cores
NCLUST = 8
BIAS = 96.0
F32 = mybir.dt.float32
F16 = mybir.dt.float16
BF16 = mybir.dt.bfloat16

PROFILE = False
LAST_RESULT = None

_PATCHED = False


def _patch_tile_drain():
    """Walrus on this image rejects multiple sync-waits on one instruction
    ("Too many sync wait commands"). Legalize by splitting surplus waits
    onto NoOp instructions inserted just before, on the same engine."""
    global _PATCHED
    if _PATCHED:
        return
    _PATCHED = True

    _orig_add = TileContext._add_instruction

    def _add_instruction(self, inst):
        si = getattr(inst, "sync_info", None)
        if (
            si is not None
            and si.on_wait
            and len(si.on_wait) > 1
            and inst.engine != mybir.EngineType.Unassigned
        ):
            waits = list(si.on_wait)
            for w in waits[:-1]:
                nop = mybir.InstNoOp(name=self.nc.get_next_instruction_name())
                nop.engine = inst.engine
                nop.sync_info = mybir.SyncInfo(on_wait=[w], on_update=[])
                _orig_add(self, nop)
            inst.sync_info = mybir.SyncInfo(
                on_wait=[waits[-1]], on_update=list(si.on_update or [])
            )
        _orig_add(self, inst)

    TileContext._add_instruction = _add_instruction

    def _drain_and_barrier(self, tick_clock, wait_clock):
        nc = self.nc
        drain_inst = nc.sync.drain()
        wait_clock.add_sem_waits(
            drain_inst.ins, ScopedClock({None: tick_clock.global_clock})
        )
        si = drain_inst.ins.sync_info
        if si is not None and si.on_wait and len(si.on_wait) > 1:
            waits = list(si.on_wait)
            drain_inst.ins.sync_info = mybir.SyncInfo(
                on_wait=waits[:1], on_update=list(si.on_update or [])
            )
            for i in range(1, len(waits)):
                nop = nc.sync.nop(nofuse=True, hint=f"drain_split_{i}")
                nop.ins.sync_info = mybir.SyncInfo(on_wait=[waits[i]], on_update=[])
        nc.all_engine_barrier()
        assert self.sems is not None
        popped = nc._tile_sem_poison_stack.pop()
        assert popped is self._sem_poison
        nc.clear_and_free_semaphores(list(self.sems.allocated().values()))
        nc.all_engine_barrier()
    TileContext._drain_and_barrier = _drain_and_barrier


def _build_nc(nslot, nu, slotmap):
    """nslot: bin-slots this core holds; nu: units (slot, 8-head group);
    slotmap[u] -> slot index. Uniform across cores (data differs)."""
    nc = bass.Bass("TRN2", target_bir_lowering=False, debug=False)

    kin = nc.dram_tensor("kin", [128, nslot * V * D], F16, kind="ExternalInput").ap()
    qin = nc.dram_tensor("qin", [D, nu * 8 * 128], F16, kind="ExternalInput").ap()
    vin = nc.dram_tensor("vin", [128, nu * 8 * 66], BF16, kind="ExternalInput").ap()
    lab = nc.dram_tensor("lab", [1, nslot * 128], F32, kind="ExternalInput").ap()
    iota8 = nc.dram_tensor("iota8", [8, 1], F32, kind="ExternalInput").ap()
    o_out = nc.dram_tensor("o", [128, nu * 8 * 65], BF16, kind="ExternalOutput").ap()

    with TileContext(nc) as tc:
        with (
            tc.tile_pool(name="singles", bufs=1) as singles,
            tc.tile_pool(name="redpool", bufs=2) as redpool,
            tc.tile_pool(name="epool", bufs=3) as epool,
            tc.tile_pool(name="ocpool", bufs=3) as ocpool,
            tc.tile_pool(name="ps_score", bufs=2, space="PSUM") as ps_score,
            tc.tile_pool(name="ps_u", bufs=4, space="PSUM") as ps_u,
        ):
            # ---- constants (no DMA deps) ----
            identity = singles.tile([128, 128], F16)
            make_identity(nc, identity)
            negb = singles.tile([128, 1], F32)
            nc.vector.memset(negb, -BIAS)
            dummy = singles.tile([128, 1], F32)
            nc.scalar.activation(dummy, negb, mybir.ActivationFunctionType.Exp)
            junk = singles.tile([128, 512], F16)
            nc.vector.memset(junk, 1.0)

            # ---- bulk DMAs: few fat 128-partition transfers.
            # lab goes first (tiny; gates the one-hot chain), then key
            # slots chained (slot s+1 starts after slot s lands) so the
            # first tree starts ASAP; q/v pieces trail the key slot they
            # pair with in consumption order. ----
            lab_sb = singles.tile([8, nslot * 128], F32)
            lab_bcast = bass.AP(tensor=lab.tensor, offset=lab.offset,
                                ap=[[0, 8]] + list(lab.ap[1:]))
            nc.sync.dma_start(out=lab_sb, in_=lab_bcast)
            iota_sb = singles.tile([8, 1], F32)
            nc.gpsimd.iota(iota_sb, [[0, 1]], channel_multiplier=1,
                           allow_small_or_imprecise_dtypes=True)

            kcs = []
            kdmas = []
            for s in range(nslot):
                kc = singles.tile([128, V * D], F16, tag=f"kc{s}", name=f"kc{s}")
                d = nc.sync.dma_start(out=kc, in_=kin[:, s * V * D:(s + 1) * V * D])
                if kdmas:
                    tile.add_dep_helper(d.ins, kdmas[-1].ins,
                                        reason="key slot chain")
                kcs.append(kc)
                kdmas.append(d)

            qtile = singles.tile([D + 8, nu * 8 * 128], F16)
            vtile = singles.tile([128, nu * 8 * 66], BF16)
            # q/v pieces paced with the key slots: units 0-1 after slot 0,
            # units 2-5 after slot 1, units 6+ after slot 2.
            qsplit = [(0, 2, 0), (2, 6, 1), (6, nu, min(2, nslot - 1))]
            for lo, hi, ks in qsplit:
                qd = nc.gpsimd.dma_start(
                    out=qtile[0:D, lo * 8 * 128:hi * 8 * 128],
                    in_=qin[:, lo * 8 * 128:hi * 8 * 128])
                tile.add_dep_helper(qd.ins, kdmas[ks].ins,
                                    reason=f"q{lo}-{hi} after ks{ks}")
                vd = nc.gpsimd.dma_start(
                    out=vtile[:, lo * 8 * 66:hi * 8 * 66],
                    in_=vin[:, lo * 8 * 66:hi * 8 * 66])
                tile.add_dep_helper(vd.ins, kdmas[ks].ins,
                                    reason=f"v{lo}-{hi} after ks{ks}")

            # ---- one-hot labels ----
            onehot = singles.tile([8, nslot * 128], F32)
            nc.vector.tensor_scalar(onehot, lab_sb, iota_sb, None,
                                    op0=mybir.AluOpType.is_equal)
            oh16 = singles.tile([8, nslot * 128], F16)
            nc.vector.tensor_copy(oh16, onehot)

            # ---- stb tiles: [S^T ; 8B*onehot] per slot ----
            stbs = []
            for s in range(nslot):
                stb = singles.tile([D + 8, 128], F16, tag=f"stb{s}",
                                   name=f"stb{s}")
                nc.vector.tensor_scalar_mul(
                    stb[D:D + 8, :], onehot[:, s * 128:(s + 1) * 128],
                    8.0 * BIAS)
                stbs.append(stb)

            # ---- PE warmup ----
            for w in range(4):
                wps = ps_score.tile([128, 512], F32, tag="ps", name=f"warm{w}")
                nc.tensor.matmul(wps, lhsT=identity, rhs=junk,
                                 start=True, stop=True)

            # ---- keysum per slot: fp16 tree -> S_s [128, D]; PE
            # transpose -> stb rows 0:D ----
            def _make_stb(s):
                cur = kcs[s]
                width = V * D
                while width > D:
                    width //= 2
                    nxt = redpool.tile([128, width], F16, tag=f"red{width}",
                                       name=f"red{s}_{width}")
                    nc.vector.tensor_tensor(
                        out=nxt, in0=cur[:, 0:width],
                        in1=cur[:, width:2 * width],
                        op=mybir.AluOpType.add)
                    cur = nxt
                st_ps = ps_score.tile([64, 128], F16, tag="ps",
                                      name=f"st{s}")
                nc.tensor.transpose(st_ps, cur, identity)
                nc.vector.tensor_copy(stbs[s][0:D, :], st_ps)

            # q one-hot rows: broadcast each slot's one-hot over that
            # slot's units and their 8 heads (DVE; fills the idle window
            # while key slot 0 is still in flight).
            qoh = qtile[D:D + 8, :].rearrange("p (u h i) -> p u h i",
                                              u=nu, h=8, i=128)
            u = 0
            while u < nu:
                s = slotmap[u]
                span = 1
                while u + span < nu and slotmap[u + span] == s:
                    span += 1
                src = oh16[:, s * 128:(s + 1) * 128]
                src_b = bass.AP(tensor=src.tensor, offset=src.offset,
                                ap=[list(src.ap[0]), [0, span], [0, 8],
                                    list(src.ap[1])])
                nc.vector.tensor_copy(qoh[:, u:u + span, :, :], src_b)
                u += span

            # slot-0 tree (gates the first scores; the qoh copies above
            # fill the DVE queue while key slot 0 is still in flight).
            _make_stb(0)
            made = 1

            # ---- unit pipeline ----
            qv = qtile.rearrange("p (u h i) -> p u h i", u=nu, h=8, i=128)
            vv = vtile.rearrange("p (u h c) -> p u h c", u=nu, h=8, c=66)
            ov = o_out.rearrange("p (u h c) -> p u h c", u=nu, h=8, c=65)

            def _ensure_stb(s):
                nonlocal made
                while made <= s:
                    _make_stb(made)
                    made += 1

            def _scores(u):
                s = slotmap[u]
                _ensure_stb(s)
                ps = ps_score.tile([128, 8, 128], F32, tag="ps",
                                   name=f"ps{u}")
                nc.tensor.matmul(ps[:, 0:4, :], lhsT=stbs[s],
                                 rhs=qv[:, u, 0:4, :], start=True, stop=True)
                nc.tensor.matmul(ps[:, 4:8, :], lhsT=stbs[s],
                                 rhs=qv[:, u, 4:8, :], start=True, stop=True)
                e_t = epool.tile([128, 8, 128], BF16, tag="et", name=f"et{u}")
                nc.scalar.activation(
                    e_t.rearrange("p a b -> p (a b)"),
                    ps.rearrange("p a b -> p (a b)"),
                    mybir.ActivationFunctionType.Exp,
                    bias=negb, scale=1.0 / 8.0)
                return e_t

            def _tail(u, e_t):
                psus = []
                for half in range(2):
                    psu = ps_u.tile([128, 4, 65], F32, tag="psu",
                                    name=f"psu{u}_{half}")
                    for hh in range(4):
                        h = half * 4 + hh
                        nc.tensor.matmul(
                            psu[:, hh, :], lhsT=e_t[:, h, :],
                            rhs=vv[:, u, h, 0:65], start=True, stop=True)
                    psus.append(psu)
                oc = ocpool.tile([128, 8, 65], BF16, tag="oc", name=f"oc{u}")
                for half in range(2):
                    nc.vector.tensor_copy(oc[:, half * 4:half * 4 + 4, :],
                                          psus[half])
                nc.scalar.dma_start(out=ov[:, u], in_=oc)

            prev = None
            for u in range(nu):
                e_t = _scores(u)
                if prev is not None:
                    _tail(*prev)
                prev = (u, e_t)
            _tail(*prev)
    return nc


_NC_CACHE = {}


def _get_nc(nslot, nu, slotmap):
    key = (nslot, nu, tuple(slotmap))
    if key not in _NC_CACHE:
        _patch_tile_drain()
        _NC_CACHE[key] = _build_nc(nslot, nu, slotmap)
    return _NC_CACHE[key]


def _pack_bins(sizes):
    """FFD pack cluster sizes into bins of <=128. Returns list of bins,
    each a list of cluster ids."""
    order = np.argsort(sizes)[::-1]
    bins, fill = [], []
    for c in order:
        placed = False
        for i in range(len(bins)):
            if fill[i] + sizes[c] <= 128:
                bins[i].append(int(c))
                fill[i] += sizes[c]
                placed = True
                break
        if not placed:
            bins.append([int(c)])
            fill.append(int(sizes[c]))
    return bins


def kernel(query, key, value, label_arr):
    """Full inputs (B,L,V,D)/(B,L) -> full output (B,L,V,D)."""
    global LAST_RESULT
    import ml_dtypes
    from concourse.bass_utils import run_bass_kernel_spmd

    query = np.asarray(query, dtype=np.float32)
    key = np.asarray(key, dtype=np.float32)
    value = np.asarray(value, dtype=np.float32)
    labels = np.asarray(label_arr)
    iota = np.arange(8, dtype=np.float32).reshape(8, 1)

    # per-batch bin packing (identical structure needed across batches)
    batch_bins = []
    for b in range(B):
        sizes = np.bincount(labels[b], minlength=NCLUST)
        assert sizes.max() <= 128, "cluster larger than 128 rows unsupported"
        batch_bins.append(_pack_bins(sizes))
    nb = max(len(bb) for bb in batch_bins)
    # units: (bin, head-group) with 4 groups of 8 heads; split 2*nb/2*nb
    # between the two cores of a batch with a uniform slot pattern.
    units_all = [(bi, g) for bi in range(nb) for g in range(4)]
    half0, half1 = units_all[:2 * nb], units_all[2 * nb:]
    if nb % 2:
        # reorder half1: full bins first, shared bin's tail groups last,
        # so both halves follow the same slot pattern (e.g. 4,4,2).
        shared = [un for un in half1 if un[0] == nb // 2]
        full = [un for un in half1 if un[0] != nb // 2]
        half1 = full + shared
    nu = 2 * nb
    # slots = unique bins in core order; slotmap uniform across halves
    def mkslots(units):
        slots, smap = [], []
        for bi, _g in units:
            if bi not in slots:
                slots.append(bi)
            smap.append(slots.index(bi))
        return slots, smap
    slots0, smap0 = mkslots(half0)
    slots1, smap1 = mkslots(half1)
    assert smap0 == smap1, (smap0, smap1)
    nslot = len(slots0)

    in_maps = []
    core_meta = []
    for c in range(NC):
        b = c // 2
        bins = batch_bins[b]
        # rows of each bin (original indices), padded to 128
        order = np.argsort(labels[b], kind="stable")
        labs_sorted = labels[b][order]
        cluster_rows = {cl: order[labs_sorted == cl] for cl in range(NCLUST)}
        binrows, binvalid = [], []
        for bb in bins:
            rows = np.concatenate([cluster_rows[cl] for cl in bb])
            w = len(rows)
            rp = np.zeros(128, dtype=np.int64)
            rp[:w] = rows
            binrows.append(rp)
            binvalid.append(w)
        while len(binrows) < nb:  # batches that packed into fewer bins
            binrows.append(np.zeros(128, dtype=np.int64))
            binvalid.append(0)

        units = half0 if c % 2 == 0 else half1
        slots = slots0 if c % 2 == 0 else slots1

        karr = np.zeros((128, nslot, V, D), dtype=np.float16)
        labarr = np.full((1, nslot * 128), -1.0, dtype=np.float32)
        for si, bi in enumerate(slots):
            w = binvalid[bi]
            if w:
                karr[:w, si] = key[b][binrows[bi][:w]]
                labarr[0, si * 128:si * 128 + w] = labels[b][binrows[bi][:w]]

        qarr = np.zeros((D, nu, 8, 128), dtype=np.float16)
        varr = np.zeros((128, nu, 8, 66), dtype=ml_dtypes.bfloat16)
        for u, (bi, g) in enumerate(units):
            w = binvalid[bi]
            if not w:
                continue
            rows = binrows[bi][:w]
            # (w, 8, D) -> (D, 8, w)
            qarr[:, u, :, :w] = query[b][rows, 8 * g:8 * g + 8, :].transpose(
                2, 1, 0)
            varr[:w, u, :, 0:D] = value[b][rows, 8 * g:8 * g + 8, :]
            varr[:w, u, :, D] = 1.0
        in_maps.append({
            "kin": karr.reshape(128, nslot * V * D),
            "qin": qarr.reshape(D, nu * 8 * 128),
            "vin": varr.reshape(128, nu * 8 * 66),
            "lab": labarr,
            "iota8": iota,
        })
        core_meta.append((b, units, binrows, binvalid))

    nc = _get_nc(nslot, nu, smap0)
    kwargs = {}
    if PROFILE:
        kwargs["trace"] = True
    res = run_bass_kernel_spmd(nc, in_maps, list(range(NC)), **kwargs)
    LAST_RESULT = res

    out = np.empty((B, L, V, D), dtype=np.float32)
    for c in range(NC):
        b, units, binrows, binvalid = core_meta[c]
        o = res.results[c]["o"].reshape(128, nu, 8, 65).astype(np.float32)
        for u, (bi, g) in enumerate(units):
            w = binvalid[bi]
            if not w:
                continue
            rows = binrows[bi][:w]
            num = o[:w, u, :, 0:D]
            den = o[:w, u, :, D:D + 1]
            out[b][rows, 8 * g:8 * g + 8, :] = num / den
    return out
